# revision 15
# baseline (speedup 1.0000x reference)
"""Causal self-attention (B=4, T=1024, D=1024, H=16) on 8 Trainium2 NeuronCores.

Sharding: heads 2c,2c+1 -> core c (head/tensor parallel). bf16 operands with
fp32 PSUM accumulation throughout.

Bias algebra (host-side folds): softmax is invariant to per-query constants,
so the k bias drops entirely; the q bias only matters through bq.k, kept by
adding bq to q during the PSUM->SBUF copy; the v bias passes through the
softmax average and folds into the output-projection bias; the 1/sqrt(dh)
scale folds into Wq. The key padding mask multiplies into the V tiles (both
the denominator ones-columns and the feature columns), so scores need no
extra mask row and the contraction stays at 64.

Head pairing: q/k live as [128, T] tiles with head0 in partitions 0:64 and
head1 in 64:128. Score matmuls for the two heads go to disjoint PE row
groups (tile_position (0,0) / (64,0)) and so run concurrently in the array;
each (key-block, query-chunk) pair writes one [128,1024] PSUM tile (head0 in
bank 0, head1 in bank 1) consumed by a single wide EXP.

PSUM layout: two pools of 2x [128,1024] fp32 tiles (8 banks). The qs pool
carries qkv accumulation groups (both 512-token halves packed -> one wide
PSUM->SBUF copy per q/k/v), paired score chunks, and proj outputs. The o
pool carries AV accumulators (both halves packed -> one reciprocal + one
normalize multiply per head) and lends slots to the PE v-transposes.

Collectives: AllToAll over batches {0,1,2} (staged incrementally as each
fin(b) completes, fired after batch 2) plus a minimal AllToAll for batch 3,
so the serial tail after the last batch is one small exchange + one 128-row
projection. Projections are emitted after the batch loop with a data
dependency on the last batch (plus a priority offset) so collective-gated
proj matmuls can never head-of-line-block batch work on the PE queue.
"""
import numpy as np

B, T, D, H = 4, 1024, 1024, 16
DH = D // H  # 64
NC = 8
HPC = H // NC  # 2 heads per core

_CACHE = {}


def _build():
    import concourse.mybir as mybir
    import concourse.tile as tile
    from concourse import bacc

    BF16 = mybir.dt.bfloat16
    F32 = mybir.dt.float32
    EXP = mybir.ActivationFunctionType.Exp
    MULT = mybir.AluOpType.mult
    ADD = mybir.AluOpType.add

    nc = bacc.Bacc("TRN2", target_bir_lowering=False, debug=False, num_devices=NC)

    xt_d = nc.dram_tensor("xt", [B, 8, 128, T], BF16, kind="ExternalInput").ap()
    wqkv_d = nc.dram_tensor("wqkv", [8, 128, 3 * 128], BF16, kind="ExternalInput").ap()
    bq_d = nc.dram_tensor("bq", [128, 1], F32, kind="ExternalInput").ap()
    maskc_d = nc.dram_tensor("maskc", [128, B * 8], F32, kind="ExternalInput").ap()
    wproj_d = nc.dram_tensor("wproj", [8, 128, D], BF16, kind="ExternalInput").ap()
    biasp_d = nc.dram_tensor("biasp", [128, D], BF16, kind="ExternalInput").ap()
    ident_d = nc.dram_tensor("ident", [128, 128], BF16, kind="ExternalInput").ap()
    tri_d = nc.dram_tensor("tri", [128, 128], BF16, kind="ExternalInput").ap()
    out_d = nc.dram_tensor("out", [B * 128, D], F32, kind="ExternalOutput").ap()

    with tile.TileContext(nc) as tc:
        with (
            tc.tile_pool(name="consts", bufs=1) as cpool,
            tc.tile_pool(name="vt", bufs=2) as vt_pool,
            tc.tile_pool(name="att", bufs=10) as att_pool,
            tc.tile_pool(name="fin", bufs=2) as fin_pool,
            tc.tile_pool(name="nrm", bufs=2) as nrm_pool,
            tc.tile_pool(name="rcv", bufs=2) as rcv_pool,
            tc.tile_pool(name="ysb", bufs=2) as y_pool,
            tc.tile_pool(name="qsps", bufs=2, space="PSUM") as qs_ps,
            tc.tile_pool(name="ops", bufs=2, space="PSUM") as o_ps_pool,
            tc.tile_pool(name="dram", bufs=1, space="DRAM") as dram,
        ):
            # ---- collective buffers (per batch; the wire time of earlier
            # exchanges hides under later batches' compute) ----
            a2a_in = [
                dram.tile([8, 128, 128], BF16, name=f"a2a_in{g}", tag=f"a2a_in{g}")
                for g in range(B)
            ]
            a2a_out = [
                dram.tile([8, 128, 128], BF16, name=f"a2a_out{g}", tag=f"a2a_out{g}")
                for g in range(B)
            ]
            # PE warm-up: ~64 tiny matmuls on a zeroed scratch tile keep the
            # PE busy from ~2us so the HAM clock gate reaches 2.4 GHz before
            # the first real matmul (and stays there)
            warm_w = cpool.tile([128, 128], BF16, name="warm", tag="warm")
            nc.vector.memset(warm_w[:], 0.0)
            for _ in range(64):
                wps = qs_ps.tile([128, 128], F32, name="wps", tag="qs")
                nc.tensor.matmul(wps[:], warm_w[:], warm_w[:], start=True, stop=True)

            # ---- constants / weights (critical-path first: wq, then x) ----
            wq_sb = cpool.tile([128, 8, 384], BF16, name="wq", tag="wq")
            nc.sync.dma_start(
                wq_sb[:, :, 0:256], wqkv_d.rearrange("a b c -> b a c")[:, :, 0:256]
            )
            nc.sync.dma_start(
                wq_sb[:, :, 256:384], wqkv_d.rearrange("a b c -> b a c")[:, :, 256:384]
            )
            maskc_sb = cpool.tile([128, B * 8], F32, name="maskc", tag="maskc")
            tri = cpool.tile([128, 128], BF16, name="tri", tag="tri")
            ident = cpool.tile([128, 128], BF16, name="ident", tag="ident")
            bq_sb = cpool.tile([128, 1], F32, name="bq", tag="bq")
            wp_sb = cpool.tile([128, 8, D], BF16, name="wp", tag="wp")
            biasp = cpool.tile([128, D], BF16, name="biasp", tag="biasp")
            ones64 = cpool.tile([128, 64], BF16, name="ones64", tag="ones64")
            nc.gpsimd.memset(ones64[:], 1.0)

            # persistent q/k tiles (two heads stacked on partitions) and v
            # tiles (ones columns for the softmax denominator), double-
            # buffered by batch parity
            qt = [None, None]
            kt = [None, None]
            for par in range(2):
                qt[par] = cpool.tile([128, T], BF16, name=f"qt{par}", tag=f"qt{par}")
                kt[par] = cpool.tile([128, T], BF16, name=f"kt{par}", tag=f"kt{par}")
            # v tiles: per head, 64 ones-columns then 64 feature columns.
            # The ones block makes the AV matmul emit 64 replicated
            # denominator rows (matmul cost depends only on the free dim),
            # so normalization needs no partition broadcast. The key padding
            # mask multiplies into both blocks (idempotent for 0/1 masks).
            v_sb = [[None] * 8, [None] * 8]
            for par in range(2):
                for kb in range(8):
                    v_t = cpool.tile(
                        [128, 320], BF16, name=f"v{par}{kb}", tag=f"v{par}{kb}"
                    )
                    nc.gpsimd.memset(v_t[:, 0:64], 1.0)
                    nc.gpsimd.memset(v_t[:, 128:192], 1.0)
                    v_sb[par][kb] = v_t

            xt_sb = [
                cpool.tile([128, 8, T], BF16, name=f"xt{par}", tag=f"xt{par}")
                for par in range(2)
            ]
            # first token-half of batch 0 arrives per d-block so the first
            # qkv accumulation group paces with the DMA stream
            for i in range(8):
                nc.sync.dma_start(xt_sb[0][:, i, 0:512], xt_d[0, i, :, 0:512])
            nc.sync.dma_start(
                xt_sb[0][:, :, 512:T], xt_d[0].rearrange("a b c -> b a c")[:, :, 512:T]
            )
            nc.sync.dma_start(maskc_sb[:], maskc_d[:])
            nc.sync.dma_start(tri[:], tri_d[:])
            nc.sync.dma_start(ident[:], ident_d[:])
            nc.sync.dma_start(bq_sb[:], bq_d[:])

            def proj_group(g):
                """Output projection for this core's 128 rows of batch g."""
                recv = rcv_pool.tile(
                    [128, 8, 128], BF16, name=f"recv{g}", tag=f"recv{g}"
                )
                # data-dependency gate on batch 3's final normalize (head 1,
                # high half -- the last write into fin): keeps the scheduler
                # from ordering proj work ahead of batch work on the PE queue
                nc.vector.tensor_copy(recv[0:1, 0, 0:2], fin_last[64:65, 1022:1024])
                nc.sync.dma_start(recv[:], a2a_out[g].rearrange("c p f -> p c f"))
                y_ps = qs_ps.tile([128, 1024], F32, name="yps", tag="qs")
                for c in range(8):
                    for ch in range(2):
                        nc.tensor.matmul(
                            y_ps[:, ch * 512 : (ch + 1) * 512],
                            recv[:, c, :],
                            wp_sb[:, c, ch * 512 : (ch + 1) * 512],
                            start=(c == 0),
                            stop=(c == 7),
                        )
                y_sb = y_pool.tile([128, D], F32, name="ysb", tag="ysb")
                nc.vector.tensor_tensor(
                    out=y_sb[:], in0=y_ps[:], in1=biasp[:], op=ADD
                )
                nc.sync.dma_start(out_d[g * 128 : (g + 1) * 128, :], y_sb[:])

            def qkv_copies(par, fb, ps, vt, csl):
                """PSUM -> persistent-tile copies for one qkv feature block."""
                if fb == 0:  # q: bias-add folded into the ACT copy
                    nc.scalar.add(qt[par][:, csl], ps[:], bq_sb[:, 0:1])
                elif fb == 1:  # k: plain copy (bias dropped)
                    nc.vector.tensor_copy(kt[par][:, csl], ps[:])
                else:  # v
                    nc.vector.tensor_copy(vt[:, csl], ps[:])

            for b in range(B):
                par = b % 2
                if b == 1:  # heavy proj-weight loads overlap batch 1
                    nc.sync.dma_start(wp_sb[:], wproj_d.rearrange("a b c -> b a c"))
                    nc.sync.dma_start(biasp[:], biasp_d[:])
                if b < B - 1:
                    nc.sync.dma_start(
                        xt_sb[1 - par][:], xt_d[b + 1].rearrange("a b c -> b a c")
                    )

                # ---- qkv projections (transposed layout [feature, token]) --
                vt = vt_pool.tile([128, T], BF16, name="vt", tag="vt")
                if b == 0:
                    # token-half-major: the first half's groups run while the
                    # second half of x streams in
                    for ch in range(2):
                        csl = slice(ch * 512, (ch + 1) * 512)
                        for fb in range(3):
                            ps = qs_ps.tile([128, 512], F32, name="qs0", tag="qs")
                            for i in range(8):
                                nc.tensor.matmul(
                                    ps[:],
                                    wq_sb[:, i, fb * 128 : (fb + 1) * 128],
                                    xt_sb[par][:, i, csl],
                                    start=(i == 0),
                                    stop=(i == 7),
                                )
                            qkv_copies(par, fb, ps, vt, csl)
                else:
                    for fb in range(3):
                        ps = qs_ps.tile([128, 1024], F32, name="qs", tag="qs")
                        for i in range(8):
                            for ch in range(2):
                                nc.tensor.matmul(
                                    ps[:, ch * 512 : (ch + 1) * 512],
                                    wq_sb[:, i, fb * 128 : (fb + 1) * 128],
                                    xt_sb[par][:, i, ch * 512 : (ch + 1) * 512],
                                    start=(i == 0),
                                    stop=(i == 7),
                                )
                        qkv_copies(par, fb, ps, vt, slice(0, T))
                # v transposed into [token, feature] blocks via PE (borrows
                # o-pool slots); the key padding mask multiplies in during
                # the PSUM->SBUF move and into the ones-columns in place
                for kb in range(8):
                    mcol = maskc_sb[:, b * 8 + kb : b * 8 + kb + 1]
                    nc.vector.tensor_scalar_mul(
                        v_sb[par][kb][:, 0:64], ones64[:], mcol
                    )
                    tp = o_ps_pool.tile([128, 128], BF16, name="tp", tag="o")
                    nc.tensor.transpose(tp[:], vt[:, kb * 128 : (kb + 1) * 128], ident[:])
                    # feature columns 64:128 (head 0) and 192:256 (head 1)
                    dst = v_sb[par][kb][:, 64:320].rearrange(
                        "p (c f) -> p c f", c=2, f=128
                    )[:, :, 0:64]
                    nc.vector.tensor_scalar_mul(dst, tp[:], mcol)

                # ---- attention: both heads together, row-group paired ----
                fin = fin_pool.tile([128, T], BF16, name="fin", tag="fin")
                att_t = []
                for kb in range(8):
                    k0 = kb * 128
                    width = T - k0
                    # [128, 2, T]: head-major attention weights
                    att = att_pool.tile([128, 2, T], BF16, name="att", tag="att")
                    for off in range(0, width, 512):
                        w = min(512, width - off)
                        s_ps = qs_ps.tile([128, 1024], F32, name="s", tag="qs")
                        for h in range(HPC):
                            # head h scores -> bank h; disjoint PE row groups
                            # run concurrently
                            nc.tensor.matmul(
                                s_ps[:, h * 512 : h * 512 + w],
                                kt[par][h * 64 : (h + 1) * 64, k0 : k0 + 128],
                                qt[par][h * 64 : (h + 1) * 64, k0 + off : k0 + off + w],
                                start=True,
                                stop=True,
                            )
                        nc.scalar.activation(
                            att[:, :, off : off + w],
                            s_ps[:].rearrange("p (h q) -> p h q", h=2)[:, :, 0:w],
                            EXP,
                        )
                    for h in range(HPC):
                        nc.vector.tensor_tensor(
                            out=att[:, h, 0:128],
                            in0=att[:, h, 0:128],
                            in1=tri[:],
                            op=MULT,
                        )
                    att_t.append(att)
                for h in range(HPC):
                    # AV accumulation (rows 0:64 of o_ps are the denominator).
                    # The low query-half only sums key blocks 0..3, so it
                    # finalizes ~7us before the last exp: normalize it early
                    # (its PSUM bank is final while the PE still accumulates
                    # the high bank), leaving only the high half on the
                    # critical path after the final exp.
                    o_ps = o_ps_pool.tile([128, 1024], F32, name="o_ps", tag="o")
                    for kb in range(4):
                        k0 = kb * 128
                        nc.tensor.matmul(
                            o_ps[:, k0:512],
                            v_sb[par][kb][:, h * 128 : (h + 1) * 128],
                            att_t[kb][:, h, 0 : 512 - k0],
                            start=(kb == 0),
                            stop=(kb == 3),
                        )
                    recip = nrm_pool.tile([64, 512], F32, name="recip", tag="recip")
                    nc.vector.reciprocal_approx_fast(recip[:], o_ps[0:64, 0:512])
                    nc.vector.tensor_tensor(
                        out=fin[h * 64 : (h + 1) * 64, 0:512],
                        in0=o_ps[64:128, 0:512],
                        in1=recip[:],
                        op=MULT,
                    )
                    for kb in range(8):
                        k0 = kb * 128
                        lo = max(k0, 512)
                        nc.tensor.matmul(
                            o_ps[:, lo:1024],
                            v_sb[par][kb][:, h * 128 : (h + 1) * 128],
                            att_t[kb][:, h, lo - k0 : 1024 - k0],
                            start=(kb == 0),
                            stop=(kb == 7),
                        )
                    recip2 = nrm_pool.tile([64, 512], F32, name="recip", tag="recip")
                    nc.vector.reciprocal_approx_fast(recip2[:], o_ps[0:64, 512:1024])
                    nc.vector.tensor_tensor(
                        out=fin[h * 64 : (h + 1) * 64, 512:1024],
                        in0=o_ps[64:128, 512:1024],
                        in1=recip2[:],
                        op=MULT,
                    )
                # stage and exchange this batch's attention outputs
                nc.sync.dma_start(
                    a2a_in[b].rearrange("c p f -> p c f"),
                    fin[:].rearrange("p (c f) -> p c f", c=8, f=128),
                )
                nc.gpsimd.collective_compute(
                    "AllToAll",
                    mybir.AluOpType.bypass,
                    replica_groups=[list(range(NC))],
                    ins=[a2a_in[b][:].opt()],
                    outs=[a2a_out[b][:].opt()],
                )
                if b == B - 1:
                    fin_last = fin
            # push priorities far past the batch pipeline so no proj work is
            # scheduled ahead of batch work on any engine queue (head-of-line)
            for g in range(B):
                tc.cur_priority += 100000
                proj_group(g)

    nc.compile()
    return nc


def _get_nc():
    if "nc" not in _CACHE:
        _CACHE["nc"] = _build()
    return _CACHE["nc"]


def kernel(x, Wqkv, bqkv, Wproj, bproj, mask):
    from concourse.bass_utils import run_bass_kernel_spmd
    import ml_dtypes

    bf16 = ml_dtypes.bfloat16
    x = np.asarray(x, dtype=np.float32)
    Wqkv = np.asarray(Wqkv, dtype=np.float32)
    bqkv = np.asarray(bqkv, dtype=np.float32)
    Wproj = np.asarray(Wproj, dtype=np.float32)
    bproj = np.asarray(bproj, dtype=np.float32)
    mask = np.asarray(mask)

    nc = _get_nc()

    xt = np.ascontiguousarray(x.transpose(0, 2, 1)).reshape(B, 8, 128, T)
    # per-key 0/1 mask columns: [key-in-block, (batch, block)]
    maskc = np.ascontiguousarray(
        (mask != 0).astype(np.float32).reshape(B, 8, 128).transpose(2, 0, 1).reshape(128, B * 8)
    )
    # v bias passes through the softmax average: fold it into the proj bias
    bproj_eff = bproj + bqkv[2 * D : 3 * D] @ Wproj
    biasp = np.broadcast_to(bproj_eff, (128, D))
    ident = np.eye(128, dtype=np.float32)
    tri = np.triu(np.ones((128, 128), np.float32))

    in_maps = []
    for c in range(NC):
        cols = slice(c * 128, (c + 1) * 128)  # this core's head features
        wq = Wqkv[:, 0:D][:, cols] * 0.125  # score scale folded into Wq
        wk = Wqkv[:, D : 2 * D][:, cols]
        wv = Wqkv[:, 2 * D : 3 * D][:, cols]
        w_local = np.concatenate([wq, wk, wv], axis=1).reshape(8, 128, 384)
        bq = (bqkv[0:D][cols] * 0.125).reshape(128, 1)
        in_maps.append(
            {
                "xt": xt.astype(bf16),
                "wqkv": np.ascontiguousarray(w_local).astype(bf16),
                "bq": np.ascontiguousarray(bq),
                "maskc": maskc,
                "wproj": Wproj.reshape(8, 128, D).astype(bf16),
                "biasp": biasp.astype(bf16),
                "ident": ident.astype(bf16),
                "tri": tri.astype(bf16),
            }
        )

    res = run_bass_kernel_spmd(nc, in_maps, core_ids=list(range(NC)))
    # core c group g rows: tokens [c*128, (c+1)*128) of batch g
    y = np.empty((B, T, D), np.float32)
    for c in range(NC):
        oc = res.results[c]["out"]
        for g in range(B):
            y[g, c * 128 : (c + 1) * 128] = oc[g * 128 : (g + 1) * 128]
    return y


# revision 17
# speedup vs baseline: 1.0990x; 1.0990x over previous
"""Causal self-attention (B=4, T=1024, D=1024, H=16) on 8 Trainium2 NeuronCores.

Sharding: heads 2c,2c+1 -> core c (head/tensor parallel). bf16 operands with
fp32 PSUM accumulation throughout.

Bias algebra (host-side folds): softmax is invariant to per-query constants,
so the k bias drops entirely; the q bias only matters through bq.k, kept by
adding bq to q during the PSUM->SBUF copy; the v bias passes through the
softmax average and folds into the output-projection bias; the 1/sqrt(dh)
scale folds into Wq. The key padding mask multiplies into the V tiles (both
the denominator ones-columns and the feature columns), so scores need no
extra mask row and the contraction stays at 64.

Head pairing: q/k live as [128, T] tiles with head0 in partitions 0:64 and
head1 in 64:128. Score matmuls for the two heads go to disjoint PE row
groups (tile_position (0,0) / (64,0)) and so run concurrently in the array;
each (key-block, query-chunk) pair writes one [128,1024] PSUM tile (head0 in
bank 0, head1 in bank 1) consumed by a single wide EXP.

PSUM layout: two pools of 2x [128,1024] fp32 tiles (8 banks). The qs pool
carries qkv accumulation groups (both 512-token halves packed -> one wide
PSUM->SBUF copy per q/k/v), paired score chunks, and proj outputs. The o
pool carries AV accumulators (both halves packed -> one reciprocal + one
normalize multiply per head) and lends slots to the PE v-transposes.

Collectives: AllToAll over batches {0,1,2} (staged incrementally as each
fin(b) completes, fired after batch 2) plus a minimal AllToAll for batch 3,
so the serial tail after the last batch is one small exchange + one 128-row
projection. Projections are emitted after the batch loop with a data
dependency on the last batch (plus a priority offset) so collective-gated
proj matmuls can never head-of-line-block batch work on the PE queue.
"""
import numpy as np

B, T, D, H = 4, 1024, 1024, 16
DH = D // H  # 64
NC = 8
HPC = H // NC  # 2 heads per core

_CACHE = {}


def _build():
    import concourse.mybir as mybir
    import concourse.tile as tile
    from concourse import bacc

    BF16 = mybir.dt.bfloat16
    F32 = mybir.dt.float32
    EXP = mybir.ActivationFunctionType.Exp
    MULT = mybir.AluOpType.mult
    ADD = mybir.AluOpType.add

    nc = bacc.Bacc("TRN2", target_bir_lowering=False, debug=False, num_devices=NC)

    xt_d = nc.dram_tensor("xt", [B, 8, 128, T], BF16, kind="ExternalInput").ap()
    wqkv_d = nc.dram_tensor("wqkv", [8, 128, 3 * 128], BF16, kind="ExternalInput").ap()
    bq_d = nc.dram_tensor("bq", [128, 1], F32, kind="ExternalInput").ap()
    maskc_d = nc.dram_tensor("maskc", [128, B * 8], F32, kind="ExternalInput").ap()
    wproj_d = nc.dram_tensor("wproj", [8, 128, D], BF16, kind="ExternalInput").ap()
    biasp_d = nc.dram_tensor("biasp", [128, D], BF16, kind="ExternalInput").ap()
    ident_d = nc.dram_tensor("ident", [128, 128], BF16, kind="ExternalInput").ap()
    tri_d = nc.dram_tensor("tri", [128, 128], BF16, kind="ExternalInput").ap()
    out_d = nc.dram_tensor("out", [B * 128, D], F32, kind="ExternalOutput").ap()

    with tile.TileContext(nc) as tc:
        with (
            tc.tile_pool(name="consts", bufs=1) as cpool,
            tc.tile_pool(name="vt", bufs=2) as vt_pool,
            tc.tile_pool(name="att", bufs=10) as att_pool,
            tc.tile_pool(name="fin", bufs=2) as fin_pool,
            tc.tile_pool(name="nrm", bufs=2) as nrm_pool,
            tc.tile_pool(name="rcv", bufs=2) as rcv_pool,
            tc.tile_pool(name="ysb", bufs=2) as y_pool,
            tc.tile_pool(name="qsps", bufs=2, space="PSUM") as qs_ps,
            tc.tile_pool(name="ops", bufs=2, space="PSUM") as o_ps_pool,
            tc.tile_pool(name="dram", bufs=1, space="DRAM") as dram,
        ):
            # ---- collective buffers (per batch; the wire time of earlier
            # exchanges hides under later batches' compute) ----
            a2a_in = [
                dram.tile([8, 128, 128], BF16, name=f"a2a_in{g}", tag=f"a2a_in{g}")
                for g in range(B)
            ]
            a2a_out = [
                dram.tile([8, 128, 128], BF16, name=f"a2a_out{g}", tag=f"a2a_out{g}")
                for g in range(B)
            ]
            # PE warm-up: ~64 tiny matmuls on a zeroed scratch tile keep the
            # PE busy from ~2us so the HAM clock gate reaches 2.4 GHz before
            # the first real matmul (and stays there)
            warm_w = cpool.tile([128, 128], BF16, name="warm", tag="warm")
            nc.vector.memset(warm_w[:], 0.0)
            for _ in range(64):
                wps = qs_ps.tile([128, 128], F32, name="wps", tag="qs")
                nc.tensor.matmul(wps[:], warm_w[:], warm_w[:], start=True, stop=True)

            # ---- constants / weights (critical-path first: wq, then x) ----
            wq_sb = cpool.tile([128, 8, 384], BF16, name="wq", tag="wq")
            nc.sync.dma_start(
                wq_sb[:, :, 0:256], wqkv_d.rearrange("a b c -> b a c")[:, :, 0:256]
            )
            nc.sync.dma_start(
                wq_sb[:, :, 256:384], wqkv_d.rearrange("a b c -> b a c")[:, :, 256:384]
            )
            maskc_sb = cpool.tile([128, B * 8], F32, name="maskc", tag="maskc")
            tri = cpool.tile([128, 128], BF16, name="tri", tag="tri")
            ident = cpool.tile([128, 128], BF16, name="ident", tag="ident")
            bq_sb = cpool.tile([128, 1], F32, name="bq", tag="bq")
            wp_sb = cpool.tile([128, 8, D], BF16, name="wp", tag="wp")
            biasp = cpool.tile([128, D], BF16, name="biasp", tag="biasp")
            ones64 = cpool.tile([128, 64], BF16, name="ones64", tag="ones64")
            nc.gpsimd.memset(ones64[:], 1.0)

            # persistent q/k tiles (two heads stacked on partitions) and v
            # tiles (ones columns for the softmax denominator), double-
            # buffered by batch parity
            qt = [None, None]
            kt = [None, None]
            for par in range(2):
                qt[par] = cpool.tile([128, T], BF16, name=f"qt{par}", tag=f"qt{par}")
                kt[par] = cpool.tile([128, T], BF16, name=f"kt{par}", tag=f"kt{par}")
            # v tiles: per head, 64 ones-columns then 64 feature columns.
            # The ones block makes the AV matmul emit 64 replicated
            # denominator rows (matmul cost depends only on the free dim),
            # so normalization needs no partition broadcast. The key padding
            # mask multiplies into both blocks (idempotent for 0/1 masks).
            v_sb = [[None] * 8, [None] * 8]
            for par in range(2):
                for kb in range(8):
                    v_t = cpool.tile(
                        [128, 320], BF16, name=f"v{par}{kb}", tag=f"v{par}{kb}"
                    )
                    nc.gpsimd.memset(v_t[:, 0:64], 1.0)
                    nc.gpsimd.memset(v_t[:, 128:192], 1.0)
                    v_sb[par][kb] = v_t

            xt_sb = [
                cpool.tile([128, 8, T], BF16, name=f"xt{par}", tag=f"xt{par}")
                for par in range(2)
            ]
            # first token-half of batch 0 arrives per d-block so the first
            # qkv accumulation group paces with the DMA stream
            for i in range(8):
                nc.sync.dma_start(xt_sb[0][:, i, 0:512], xt_d[0, i, :, 0:512])
            nc.sync.dma_start(
                xt_sb[0][:, :, 512:T], xt_d[0].rearrange("a b c -> b a c")[:, :, 512:T]
            )
            nc.sync.dma_start(maskc_sb[:], maskc_d[:])
            nc.sync.dma_start(tri[:], tri_d[:])
            nc.sync.dma_start(ident[:], ident_d[:])
            nc.sync.dma_start(bq_sb[:], bq_d[:])

            def proj_group(g):
                """Output projection for this core's 128 rows of batch g."""
                recv = rcv_pool.tile(
                    [128, 8, 128], BF16, name=f"recv{g}", tag=f"recv{g}"
                )
                # data-dependency gate on batch 3's final normalize (head 1
                # is written last): keeps the scheduler from ordering proj
                # work ahead of batch work on the PE queue
                nc.vector.tensor_copy(recv[0:1, 0, 0:2], fin_last[64:65, 1022:1024])
                nc.sync.dma_start(recv[:], a2a_out[g].rearrange("c p f -> p c f"))
                y_ps = qs_ps.tile([128, 1024], F32, name="yps", tag="qs")
                for c in range(8):
                    for ch in range(2):
                        nc.tensor.matmul(
                            y_ps[:, ch * 512 : (ch + 1) * 512],
                            recv[:, c, :],
                            wp_sb[:, c, ch * 512 : (ch + 1) * 512],
                            start=(c == 0),
                            stop=(c == 7),
                        )
                y_sb = y_pool.tile([128, D], F32, name="ysb", tag="ysb")
                nc.vector.tensor_tensor(
                    out=y_sb[:], in0=y_ps[:], in1=biasp[:], op=ADD
                )
                nc.sync.dma_start(out_d[g * 128 : (g + 1) * 128, :], y_sb[:])

            def qkv_copies(par, fb, ps, vt, csl):
                """PSUM -> persistent-tile copies for one qkv feature block."""
                if fb == 0:  # q: bias-add folded into the ACT copy
                    nc.scalar.add(qt[par][:, csl], ps[:], bq_sb[:, 0:1])
                elif fb == 1:  # k: plain copy (bias dropped)
                    nc.vector.tensor_copy(kt[par][:, csl], ps[:])
                else:  # v
                    nc.vector.tensor_copy(vt[:, csl], ps[:])

            for b in range(B):
                par = b % 2
                if b == 1:  # heavy proj-weight loads overlap batch 1
                    nc.sync.dma_start(wp_sb[:], wproj_d.rearrange("a b c -> b a c"))
                    nc.sync.dma_start(biasp[:], biasp_d[:])
                if b < B - 1:
                    nc.sync.dma_start(
                        xt_sb[1 - par][:], xt_d[b + 1].rearrange("a b c -> b a c")
                    )

                # ---- qkv projections (transposed layout [feature, token]) --
                vt = vt_pool.tile([128, T], BF16, name="vt", tag="vt")
                if b == 0:
                    # token-half-major: the first half's groups run while the
                    # second half of x streams in
                    for ch in range(2):
                        csl = slice(ch * 512, (ch + 1) * 512)
                        for fb in range(3):
                            ps = qs_ps.tile([128, 512], F32, name="qs0", tag="qs")
                            for i in range(8):
                                nc.tensor.matmul(
                                    ps[:],
                                    wq_sb[:, i, fb * 128 : (fb + 1) * 128],
                                    xt_sb[par][:, i, csl],
                                    start=(i == 0),
                                    stop=(i == 7),
                                )
                            qkv_copies(par, fb, ps, vt, csl)
                else:
                    for fb in range(3):
                        ps = qs_ps.tile([128, 1024], F32, name="qs", tag="qs")
                        for i in range(8):
                            for ch in range(2):
                                nc.tensor.matmul(
                                    ps[:, ch * 512 : (ch + 1) * 512],
                                    wq_sb[:, i, fb * 128 : (fb + 1) * 128],
                                    xt_sb[par][:, i, ch * 512 : (ch + 1) * 512],
                                    start=(i == 0),
                                    stop=(i == 7),
                                )
                        qkv_copies(par, fb, ps, vt, slice(0, T))
                # v transposed into [token, feature] blocks via PE (borrows
                # o-pool slots); the key padding mask multiplies in during
                # the PSUM->SBUF move and into the ones-columns in place
                for kb in range(8):
                    mcol = maskc_sb[:, b * 8 + kb : b * 8 + kb + 1]
                    nc.vector.tensor_scalar_mul(
                        v_sb[par][kb][:, 0:64], ones64[:], mcol
                    )
                    tp = o_ps_pool.tile([128, 128], BF16, name="tp", tag="o")
                    nc.tensor.transpose(tp[:], vt[:, kb * 128 : (kb + 1) * 128], ident[:])
                    # feature columns 64:128 (head 0) and 192:256 (head 1)
                    dst = v_sb[par][kb][:, 64:320].rearrange(
                        "p (c f) -> p c f", c=2, f=128
                    )[:, :, 0:64]
                    nc.vector.tensor_scalar_mul(dst, tp[:], mcol)

                # ---- attention: both heads together, row-group paired ----
                fin = fin_pool.tile([128, T], BF16, name="fin", tag="fin")
                att_t = []
                for kb in range(8):
                    k0 = kb * 128
                    width = T - k0
                    # [128, 2, T]: head-major attention weights
                    att = att_pool.tile([128, 2, T], BF16, name="att", tag="att")
                    for off in range(0, width, 512):
                        w = min(512, width - off)
                        s_ps = qs_ps.tile([128, 1024], F32, name="s", tag="qs")
                        for h in range(HPC):
                            # head h scores -> bank h; disjoint PE row groups
                            # run concurrently
                            nc.tensor.matmul(
                                s_ps[:, h * 512 : h * 512 + w],
                                kt[par][h * 64 : (h + 1) * 64, k0 : k0 + 128],
                                qt[par][h * 64 : (h + 1) * 64, k0 + off : k0 + off + w],
                                start=True,
                                stop=True,
                            )
                        nc.scalar.activation(
                            att[:, :, off : off + w],
                            s_ps[:].rearrange("p (h q) -> p h q", h=2)[:, :, 0:w],
                            EXP,
                        )
                    for h in range(HPC):
                        nc.vector.tensor_tensor(
                            out=att[:, h, 0:128],
                            in0=att[:, h, 0:128],
                            in1=tri[:],
                            op=MULT,
                        )
                    att_t.append(att)
                for h in range(HPC):
                    # AV accumulation (rows 0:64 of o_ps are the denominator)
                    o_ps = o_ps_pool.tile([128, 1024], F32, name="o_ps", tag="o")
                    for kb in range(8):
                        k0 = kb * 128
                        if k0 < 512:
                            nc.tensor.matmul(
                                o_ps[:, k0:512],
                                v_sb[par][kb][:, h * 128 : (h + 1) * 128],
                                att_t[kb][:, h, 0 : 512 - k0],
                                start=(kb == 0),
                                stop=(kb == 3),
                            )
                        lo = max(k0, 512)
                        nc.tensor.matmul(
                            o_ps[:, lo:1024],
                            v_sb[par][kb][:, h * 128 : (h + 1) * 128],
                            att_t[kb][:, h, lo - k0 : 1024 - k0],
                            start=(kb == 0),
                            stop=(kb == 7),
                        )
                    # normalize: rows 0:64 of o_ps are 64 copies of the
                    # denominator, rows 64:128 the features
                    recip = nrm_pool.tile([64, 1024], F32, name="recip", tag="recip")
                    nc.vector.reciprocal_approx_fast(recip[:], o_ps[0:64, :])
                    nc.vector.tensor_tensor(
                        out=fin[h * 64 : (h + 1) * 64, :],
                        in0=o_ps[64:128, :],
                        in1=recip[:],
                        op=MULT,
                    )
                # stage and exchange this batch's attention outputs
                nc.sync.dma_start(
                    a2a_in[b].rearrange("c p f -> p c f"),
                    fin[:].rearrange("p (c f) -> p c f", c=8, f=128),
                )
                nc.gpsimd.collective_compute(
                    "AllToAll",
                    mybir.AluOpType.bypass,
                    replica_groups=[list(range(NC))],
                    ins=[a2a_in[b][:].opt()],
                    outs=[a2a_out[b][:].opt()],
                )
                if b == B - 1:
                    fin_last = fin
            # push priorities far past the batch pipeline so no proj work is
            # scheduled ahead of batch work on any engine queue (head-of-line)
            for g in range(B):
                tc.cur_priority += 100000
                proj_group(g)

    nc.compile()
    return nc


def _get_nc():
    if "nc" not in _CACHE:
        _CACHE["nc"] = _build()
    return _CACHE["nc"]


def kernel(x, Wqkv, bqkv, Wproj, bproj, mask):
    from concourse.bass_utils import run_bass_kernel_spmd
    import ml_dtypes

    bf16 = ml_dtypes.bfloat16
    x = np.asarray(x, dtype=np.float32)
    Wqkv = np.asarray(Wqkv, dtype=np.float32)
    bqkv = np.asarray(bqkv, dtype=np.float32)
    Wproj = np.asarray(Wproj, dtype=np.float32)
    bproj = np.asarray(bproj, dtype=np.float32)
    mask = np.asarray(mask)

    nc = _get_nc()

    xt = np.ascontiguousarray(x.transpose(0, 2, 1)).reshape(B, 8, 128, T)
    # per-key 0/1 mask columns: [key-in-block, (batch, block)]
    maskc = np.ascontiguousarray(
        (mask != 0).astype(np.float32).reshape(B, 8, 128).transpose(2, 0, 1).reshape(128, B * 8)
    )
    # v bias passes through the softmax average: fold it into the proj bias
    bproj_eff = bproj + bqkv[2 * D : 3 * D] @ Wproj
    biasp = np.broadcast_to(bproj_eff, (128, D))
    ident = np.eye(128, dtype=np.float32)
    tri = np.triu(np.ones((128, 128), np.float32))

    in_maps = []
    for c in range(NC):
        cols = slice(c * 128, (c + 1) * 128)  # this core's head features
        wq = Wqkv[:, 0:D][:, cols] * 0.125  # score scale folded into Wq
        wk = Wqkv[:, D : 2 * D][:, cols]
        wv = Wqkv[:, 2 * D : 3 * D][:, cols]
        w_local = np.concatenate([wq, wk, wv], axis=1).reshape(8, 128, 384)
        bq = (bqkv[0:D][cols] * 0.125).reshape(128, 1)
        in_maps.append(
            {
                "xt": xt.astype(bf16),
                "wqkv": np.ascontiguousarray(w_local).astype(bf16),
                "bq": np.ascontiguousarray(bq),
                "maskc": maskc,
                "wproj": Wproj.reshape(8, 128, D).astype(bf16),
                "biasp": biasp.astype(bf16),
                "ident": ident.astype(bf16),
                "tri": tri.astype(bf16),
            }
        )

    res = run_bass_kernel_spmd(nc, in_maps, core_ids=list(range(NC)))
    # core c group g rows: tokens [c*128, (c+1)*128) of batch g
    y = np.empty((B, T, D), np.float32)
    for c in range(NC):
        oc = res.results[c]["out"]
        for g in range(B):
            y[g, c * 128 : (c + 1) * 128] = oc[g * 128 : (g + 1) * 128]
    return y
